# revision 74
# baseline (speedup 1.0000x reference)
"""Trainium2 Bass kernel for nn_Decoder_38757784879455 (GNN message passing).

Sparse-compaction design (8 cores, 4 scenes/core, data-parallel over scenes),
with a software-pipelined per-scene schedule:

  * scale-free geometry (no sqrt/recip; one act-table load total): scores
    [i, j] = mask * (64 - j), computed per scene so scene 0 unblocks early
  * two DVE max8/max_index rounds -> top-12 neighbour indices per i
  * index row replicated across partitions with ONE broadcast-read DMA
    (DRAM -> (P, PK) stride-0 partition AP); one-hot S built by is_equal
  * all matmul operands bf16; beff folded into the statq matmul via a
    ones contraction row so the h1 relus are bias-free
  * h1pre on the PE: stationary [statq_top; statq_geo], moving [S; -eye x 1K]
  * h2 = relu(h1) @ Wm2 over 64*K pairs; relu+bm2 applied on the psum->sbuf
    copy (monotonicity), pooling via bf16 tensor_tensor max/min trees
    (12 -> 6 -> 3 -> 1) which get DVE fast modes
  * emission skew: selection (front_a) runs 2-3 scenes ahead of the MLP
    (front_b); pooling+output (back) of scene s-1 lands after front_b(s)
    so neither the PE nor the DVE head-of-line blocks on same-scene deps
  * queues: SP carries geopack/wstat + the per-scene index roundtrip;
    Pool's SWDGE carries the bulk weight loads; Act issues no DMAs
  * HW constraints honoured: GPSIMD touches no PSUM and runs only
    tensor_tensor add/sub/mult, partition_broadcast, memset, DMA
"""

import math

import numpy as np
import ml_dtypes

import concourse.bass as bass
import concourse.mybir as mybir
import concourse.tile as tile
from concourse import bacc
from concourse.bass_utils import run_bass_kernel_spmd

# problem constants
E = 64
H = 128
D = 256
MLP = 512
B_SEQ = 32
P = 64
N = B_SEQ * P
NCORES = 8
S = B_SEQ // NCORES          # scenes per core
NP_CORE = S * P              # pedestrians per core
K = 12                       # neighbour slots per pedestrian (max count is 12)
PK = P * K                   # compacted pairs per scene

DEG_VISION = 120.0
_half = DEG_VISION / 2.0
BCONE = math.sin(math.radians(_half)) * (2.0 / math.cos(math.radians(_half)))

FP = mybir.dt.float32
BF = mybir.dt.bfloat16
F8 = mybir.dt.float8e4
U32 = mybir.dt.uint32
ALU = mybir.AluOpType
ACTF = mybir.ActivationFunctionType
AX = mybir.AxisListType

# power-of-2 scales so fp8e4 operands sit in their full-mantissa range
H1SC = 8.0       # relu(h1) stored as 8*h1 in fp8
WM2SC = 64.0     # Wm2 stored as 64*Wm2 in fp8
H2UNSC = 1.0 / (H1SC * WM2SC)

# ---- geopack (f32, 128 x GCOLS) column layout ----
# geometry deps first (head), output-stage data second (tail);
# the [px_s | py_s] broadcast row rides in its own tiny (1, 2*NP) tensor
G_GEOT = 0                       # (P, 8*S) per-pedestrian geometry cols
G_IOTA = G_GEOT + 8 * S          # (P, P)   (64-j)*(i!=j) score weights
G_IOTJ = G_IOTA + P              # (P, 1)   partition index column
G_HEAD = G_IOTJ + 1              # end of geometry head
G_RBP = G_HEAD                   # (P, D)   relu(bp) replicated
G_BM2 = G_RBP + D                # (128, 2) bm2 (per m2 column)
G_PXY = G_BM2 + 2                # (1, 2*NP_CORE) [px_s | py_s] rows
GCOLS = G_PXY + 2 * NP_CORE
GCOLS = ((GCOLS + 127) // 128) * 128

# ---- wpack (bf16, 128 x WCOLS) column layout ----
# order chosen so the statq deps (hidT/a4/wm1h/g4) load first
W_HIDT = 0                       # (128, NP_CORE) hid^T
W_A4 = W_HIDT + NP_CORE          # (4, MLP)   folded pos/vel weights
W_WM1H = W_A4 + MLP              # (128, MLP) Wm1[hid part]
W_G4 = W_WM1H + MLP              # (4, NP_CORE) px,py,vx,vy rows (bf16)
W_WM2 = W_G4 + NP_CORE           # (128, 4*D) Wm2 packed by kc
W_WP = W_WM2 + 4 * D             # (128, 4*D) Wp packed by kc
W_BP = W_WP + 4 * D              # (1, D)     bp row
W_ONES = W_BP + D                # (1, P)     ones row
W_EYE = W_ONES + P               # (P, PK)    -eye x 1_K block (DMA'd from
                                 #            DRAM per scene; not kept in SBUF)
WCOLS = W_EYE + PK
WCOLS = ((WCOLS + 127) // 128) * 128


def build_program(reps=1):
    """Per-core Bass program (same program on all 8 cores)."""
    nc = bacc.Bacc(None, target_bir_lowering=False, debug=False)

    geopack = nc.dram_tensor("geopack", [128, GCOLS], FP, kind="ExternalInput").ap()
    wpack = nc.dram_tensor("wpack", [128, WCOLS], BF, kind="ExternalInput").ap()
    outp = nc.dram_tensor("outp", [NP_CORE, D], FP, kind="ExternalOutput").ap()

    with tile.TileContext(nc) as tc:
        with (
            tc.tile_pool(name="singles", bufs=1) as singles,
            tc.tile_pool(name="geom", bufs=1) as geom,
            tc.tile_pool(name="topk", bufs=3) as topk,
            tc.tile_pool(name="idxr", bufs=3) as idxrp,
            tc.tile_pool(name="sel", bufs=2) as selp,
            tc.tile_pool(name="h1", bufs=8) as h1p,
            tc.tile_pool(name="small", bufs=4) as small,
            tc.tile_pool(name="outs", bufs=2) as outsp,
            tc.tile_pool(name="psQ", bufs=1, space="PSUM") as psQ,
            tc.tile_pool(name="psH1", bufs=2, space="PSUM") as psH1,
            tc.tile_pool(name="psH2", bufs=1, space="PSUM") as psH2,
            tc.tile_pool(name="psO", bufs=1, space="PSUM") as psO,
            tc.tile_pool(name="dram", bufs=3, space="DRAM") as dramp,
        ):
            # ---- prologue: parallel loads on separate queues ----
            # sync (SP) queue: geometry head then statq weight deps -- the
            # critical-path loads. scalar (Act) queue (delayed ~1.3us by the
            # act-table load): sext static halves, mov weights, small tails.
            geos = singles.tile([128, GCOLS], FP)
            nc.sync.dma_start(out=geos[:], in_=geopack[:])
            wsb = singles.tile([128, W_EYE], BF)
            nc.sync.dma_start(out=wsb[:, 0:W_WM2], in_=wpack[:, 0:W_WM2])
            sexts = [singles.tile([128, PK], BF, name=f"sext{i}") for i in range(2)]

            geoT_sb = geos[0:P, G_GEOT : G_GEOT + 8 * S]
            iota_sb = geos[0:P, G_IOTA : G_IOTA + P]
            iotj_sb = geos[0:P, G_IOTJ : G_IOTJ + 1]
            rbp_sb = geos[0:P, G_RBP : G_RBP + D]
            bm2_sb = geos[:, G_BM2 : G_BM2 + 2]

            out2all = singles.tile([P, S * D], FP, name="out2all")
            hidT_bf = wsb[0:H, W_HIDT : W_HIDT + NP_CORE]
            a5_bf = wsb[0:5, W_A4 : W_A4 + MLP]
            wm1h_bf = wsb[0:H, W_WM1H : W_WM1H + MLP]
            wm2_bf = wsb[:, W_WM2 : W_WM2 + 4 * D]
            wp_bf = wsb[:, W_WP : W_WP + 4 * D]
            bp_bf = wsb[0:1, W_BP : W_BP + D]
            ones_bf = wsb[0:1, W_ONES : W_ONES + P]
            g5_bf = wsb[0:5, W_G4 : W_G4 + NP_CORE]

            for rep in range(reps):
                # ======== geometry (scale-free: no sqrt/reciprocal) ========
                # With u = (xb-x, yb-y): X = r*x_t = yr*dx + xr*dy (only ever
                # squared, sign-free), Y = r*y_t = xr*dx - yr*dy.
                # egg:  X^2 + 0.25*Y^2 <= r2 ; cone: B^2*Y^2 > 4*X^2 ; Y >= 0.
                gcol = lambda r: geoT_sb[:, r :: 8]          # (P, S) strided
                xr = geom.tile([P, S], FP, name="xr")
                nc.vector.tensor_tensor(out=xr[:], in0=gcol(4), in1=gcol(0), op=ALU.subtract)
                yr = geom.tile([P, S], FP, name="yr")
                nc.vector.tensor_tensor(out=yr[:], in0=gcol(5), in1=gcol(1), op=ALU.subtract)
                xrh = geom.tile([P, S], FP, name="xrh")
                nc.vector.tensor_scalar(out=xrh[:], in0=xr[:], scalar1=0.5, scalar2=None, op0=ALU.mult)
                yrh = geom.tile([P, S], FP, name="yrh")
                nc.vector.tensor_scalar(out=yrh[:], in0=yr[:], scalar1=0.5, scalar2=None, op0=ALU.mult)
                r2 = geom.tile([P, S], FP, name="r2")
                nc.vector.tensor_tensor(out=r2[:], in0=xr[:], in1=xr[:], op=ALU.mult)
                yr2 = geom.tile([P, S], FP, name="yr2")
                nc.vector.tensor_tensor(out=yr2[:], in0=yr[:], in1=yr[:], op=ALU.mult)
                nc.vector.tensor_tensor(out=r2[:], in0=r2[:], in1=yr2[:], op=ALU.add)

                # ======== per-scene pipeline (geometry inlined per scene so
                # scene 0's selection unblocks the PE as early as possible;
                # the tail (pool-trees + output) of scene s is EMITTED after
                # scene s+1's front so the PE never head-of-line blocks on
                # same-scene pooling) ====
                iob = iota_sb.rearrange("p (a j) -> p a j", a=1).to_broadcast([P, 1, P])
                tails = []

                def front_a(s):
                    c0 = s * P
                    gc = lambda r: geoT_sb[0:P, s * 8 + r : s * 8 + r + 1]
                    def b3(t):  # (P,1) -> broadcast (P,1,P)
                        return t.rearrange("p (s j) -> p s j", j=1).to_broadcast([P, 1, P])
                    xrc, yrc = xr[:, s : s + 1], yr[:, s : s + 1]
                    xrhc, yrhc = xrh[:, s : s + 1], yrh[:, s : s + 1]
                    r2c = r2[:, s : s + 1]
                    # pj[:,0,:] = px_j, pj[:,1,:] = py_j (Pool bcast from the
                    # host-packed [px_s | py_s] row)
                    pj = geom.tile([P, 2, P], FP, name=f"pj{s}", bufs=1)
                    nc.gpsimd.partition_broadcast(
                        out_ap=pj[:].rearrange("p a j -> p (a j)"),
                        in_ap=geos[0:1, G_PXY + s * 2 * P : G_PXY + (s + 1) * 2 * P])
                    pjx, pjy = pj[:, 0:1, :], pj[:, 1:2, :]
                    dx = geom.tile([P, 1, P], FP, name=f"dx{s}", bufs=1)
                    nc.vector.tensor_tensor(out=dx[:], in0=pjx, in1=b3(gc(0)), op=ALU.subtract)
                    dy = geom.tile([P, 1, P], FP, name=f"dy{s}", bufs=1)
                    nc.vector.tensor_tensor(out=dy[:], in0=pjy, in1=b3(gc(1)), op=ALU.subtract)
                    t1 = geom.tile([P, 1, P], FP, name=f"t1{s}", bufs=1)
                    nc.vector.tensor_tensor(out=t1[:], in0=dx[:], in1=b3(yrc), op=ALU.mult)
                    t2 = geom.tile([P, 1, P], FP, name=f"t2{s}", bufs=1)
                    nc.vector.tensor_tensor(out=t2[:], in0=dy[:], in1=b3(xrc), op=ALU.mult)
                    xb = geom.tile([P, 1, P], FP, name=f"xb{s}", bufs=1)
                    nc.vector.tensor_tensor(out=xb[:], in0=t1[:], in1=t2[:], op=ALU.add)
                    x2 = geom.tile([P, 1, P], FP, name=f"x2{s}", bufs=1)
                    nc.vector.tensor_tensor(out=x2[:], in0=xb[:], in1=xb[:], op=ALU.mult)
                    t3 = geom.tile([P, 1, P], FP, name=f"t3{s}", bufs=1)
                    nc.vector.tensor_tensor(out=t3[:], in0=dx[:], in1=b3(xrhc), op=ALU.mult)
                    t4 = geom.tile([P, 1, P], FP, name=f"t4{s}", bufs=1)
                    nc.vector.tensor_tensor(out=t4[:], in0=dy[:], in1=b3(yrhc), op=ALU.mult)
                    yb = geom.tile([P, 1, P], FP, name=f"yb{s}", bufs=1)
                    nc.vector.tensor_tensor(out=yb[:], in0=t3[:], in1=t4[:], op=ALU.subtract)
                    y2 = geom.tile([P, 1, P], FP, name=f"y2{s}", bufs=1)
                    nc.vector.tensor_tensor(out=y2[:], in0=yb[:], in1=yb[:], op=ALU.mult)
                    # yb = 0.5*Y, y2 = 0.25*Y^2; egg: x2 + 0.25*Y^2 <= r2
                    res = geom.tile([P, 1, P], FP, name=f"res{s}", bufs=1)
                    nc.vector.tensor_tensor(out=res[:], in0=y2[:], in1=x2[:], op=ALU.add)
                    e2 = geom.tile([P, 1, P], FP, name=f"e2{s}", bufs=1)
                    nc.vector.tensor_tensor(out=e2[:], in0=b3(r2c), in1=res[:], op=ALU.subtract)
                    # g = min(e2, 0.5*Y): valid needs g >= 0 and cone > 0
                    g = geom.tile([P, 1, P], FP, name=f"g{s}", bufs=1)
                    nc.vector.tensor_tensor(out=g[:], in0=e2[:], in1=yb[:], op=ALU.min)
                    # cone: B^2*Y^2 > 4*X^2  <=>  0.25*Y^2 > x2/B^2
                    cv = geom.tile([P, 1, P], FP, name=f"cv{s}", bufs=1)
                    nc.vector.scalar_tensor_tensor(out=cv[:], in0=x2[:], scalar=-1.0 / (BCONE * BCONE), in1=y2[:], op0=ALU.mult, op1=ALU.add)
                    cvi = geom.tile([P, 1, P], FP, name=f"cvi{s}", bufs=1)
                    nc.vector.scalar_tensor_tensor(out=cvi[:], in0=cv[:], scalar=0.0, in1=iob, op0=ALU.is_gt, op1=ALU.mult)
                    sch = geom.tile([P, 1, P], FP, name=f"sc{s}", bufs=1)
                    nc.vector.scalar_tensor_tensor(out=sch[:], in0=g[:], scalar=0.0, in1=cvi[:], op0=ALU.is_ge, op1=ALU.mult)
                    sc_s = sch[:, 0, :]
                    sc_s = sc_s.rearrange("p a j -> p (a j)") \
                        if len(sc_s.shape) > 2 else sc_s

                    # ---- top-K selection ----
                    v8 = topk.tile([P, 16], FP, name="v8")
                    nc.vector.max(out=v8[:, 0:8], in_=sc_s)
                    idxc = topk.tile([P, 16], U32, name="idxc")
                    nc.vector.max_index(out=idxc[:, 0:8], in_max=v8[:, 0:8], in_values=sc_s)
                    lt = topk.tile([P, P], FP, name="lt")
                    nc.vector.tensor_scalar(out=lt[:], in0=sc_s, scalar1=v8[:, 7:8], scalar2=None, op0=ALU.is_lt)
                    sc2 = topk.tile([P, P], FP, name="sc2")
                    nc.vector.tensor_tensor(out=sc2[:], in0=sc_s, in1=lt[:], op=ALU.mult)
                    nc.vector.max(out=v8[:, 8:16], in_=sc2[:])
                    nc.vector.max_index(out=idxc[:, 8:16], in_max=v8[:, 8:16], in_values=sc2[:])

                    has = small.tile([P, 1], FP, name="has", bufs=6)
                    nc.vector.tensor_scalar(out=has[:], in0=v8[:, 0:1], scalar1=0.0, scalar2=None, op0=ALU.is_gt)
                    hinv = small.tile([P, 1], FP, name="hinv", bufs=6)
                    nc.vector.tensor_scalar(out=hinv[:], in0=has[:], scalar1=-1.0, scalar2=1.0, op0=ALU.mult, op1=ALU.add)

                    # fixup: empty slots -> duplicate slot-0 index
                    idxf32 = topk.tile([P, 16], FP, name="idxf32")
                    nc.vector.tensor_copy(out=idxf32[:], in_=idxc[:])
                    vs = topk.tile([P, 16], FP, name="vs")
                    nc.vector.tensor_scalar(out=vs[:], in0=v8[:], scalar1=0.0, scalar2=None, op0=ALU.is_gt)
                    dv = topk.tile([P, 16], FP, name="dv")
                    nc.vector.scalar_tensor_tensor(
                        out=dv[:], in0=idxf32[:], scalar=idxf32[:, 0:1], in1=vs[:],
                        op0=ALU.subtract, op1=ALU.mult)
                    idxbf = topk.tile([P, 16], BF, name="idxbf")
                    nc.vector.tensor_scalar(out=idxbf[:], in0=dv[:], scalar1=idxf32[:, 0:1], scalar2=None, op0=ALU.add)

                    # ---- index replication: DRAM roundtrip + bcast read ----
                    idr = dramp.tile([P, K], BF, name="idr")
                    nc.sync.dma_start(out=idr[:], in_=idxbf[:, 0:K])
                    idxrep = idxrp.tile([P, PK], BF, name="idxrep")
                    nc.sync.dma_start(out=idxrep[:], in_=bass.AP(
                        tensor=idr[:].tensor, offset=idr[:].offset,
                        ap=[[0, P], [1, PK]]))
                    return {"s": s, "idxrep": idxrep, "has": has, "hinv": hinv}

                def statq_mm(s):
                    # statq: [top = geo*A4 + beff + hid*Wm1h ; bottom =
                    # geo*A4] (beff folded in via the ones row of g5/a5)
                    c0 = s * P
                    ps_q = psQ.tile([128, MLP], FP, tag="psq", name="psq")
                    nc.tensor.matmul(ps_q[0:P, :], g5_bf[:, c0 : c0 + P], a5_bf, start=True, stop=False)
                    nc.tensor.matmul(ps_q[0:P, :], hidT_bf[:, c0 : c0 + P], wm1h_bf, start=False, stop=True)
                    nc.tensor.matmul(ps_q[P : 2 * P, :], g5_bf[0:4, c0 : c0 + P], a5_bf[0:4, :], start=True, stop=True)
                    statq = selp.tile([128, MLP], BF, name="statq")
                    nc.scalar.activation(out=statq[:], in_=ps_q[:], func=ACTF.Copy)
                    return statq

                def front_b(st, statq, statq_next):
                    s = st["s"]
                    c0 = s * P
                    # ---- one-hot S_ext top half (bottom -eye x 1K is static,
                    # loaded once into both buffers in the prologue) ----
                    sext = sexts[s % 2]
                    nc.vector.tensor_scalar(
                        out=sext[0:P, :], in0=st["idxrep"][:],
                        scalar1=iotj_sb, scalar2=None, op0=ALU.is_equal)

                    # ---- h1 matmuls (chunked by PSUM bank) + ONE fused
                    # bias-free relu per mt over the full 768 cols ----
                    CH = [(c, min(512, PK - c)) for c in range(0, PK, 512)]
                    h1f = []
                    for mt in range(4):
                        h1t = h1p.tile([128, PK], BF, tag="h1f", name="h1f")
                        ps_h1 = psH1.tile([128, PK], FP, tag="psh1", name="psh1")
                        for c0h, w in CH:
                            nc.tensor.matmul(
                                ps_h1[:, c0h : c0h + w],
                                statq[:, mt * 128 : (mt + 1) * 128],
                                sext[:, c0h : c0h + w],
                                start=True, stop=True)
                        # balance: Act gets mt0-mt2, DVE mt3 (GPSIMD cannot
                        # read PSUM on real HW)
                        if mt < 3:
                            nc.scalar.activation(
                                out=h1t[:], in_=ps_h1[:], func=ACTF.Relu)
                        else:
                            nc.vector.tensor_scalar(
                                out=h1t[:], in0=ps_h1[:], scalar1=0.0,
                                scalar2=None, op0=ALU.max)
                        h1f.append(h1t)

                    # prefetch next scene's statq while this scene's h2 runs
                    if statq_next is not None:
                        statq_next.append(statq_mm(statq_next.pop()))

                    # ---- h2 matmuls; relu+bm2 (true h2) on the psum->sbuf
                    # copy ----
                    h2bfs = []
                    for m2 in range(2):
                        ps_h2 = psH2.tile([128, PK], FP, tag="psh2", name="psh2")
                        for kc in range(4):
                            for c0h, w in CH:
                                nc.tensor.matmul(
                                    ps_h2[:, c0h : c0h + w],
                                    wm2_bf[:, kc * D + m2 * 128 : kc * D + (m2 + 1) * 128],
                                    h1f[kc][:, c0h : c0h + w],
                                    start=(kc == 0), stop=(kc == 3))
                        h2bf = small.tile([128, PK], BF, name=f"h2bf{m2}", bufs=2)
                        nc.scalar.activation(
                            out=h2bf[:], in_=ps_h2[:], func=ACTF.Relu,
                            bias=bm2_sb[:, m2 : m2 + 1])
                        h2bfs.append(h2bf)
                    return {"s": s, "h2bfs": h2bfs, "has": st["has"], "hinv": st["hinv"]}

                def back(st):
                    s = st["s"]
                    c0 = s * P
                    # max/min over the K=12 slot groups via bf16 tt trees
                    # (12 -> 6 -> 3 -> 1); tt gets DVE fast modes, reduce
                    # doesn't
                    pooled = [None] * 4
                    for m2 in range(2):
                        hv = st["h2bfs"][m2][:].rearrange("p (i k) -> p i k", k=K)
                        for r, op in ((0, ALU.max), (2, ALU.min)):
                            a1 = small.tile([128, P, 6], BF, name=f"a1_{m2}_{r}", bufs=2)
                            nc.vector.tensor_tensor(
                                out=a1[:], in0=hv[:, :, 0:6], in1=hv[:, :, 6:12], op=op)
                            a2 = small.tile([128, P, 3], BF, name=f"a2_{m2}_{r}", bufs=2)
                            nc.vector.tensor_tensor(
                                out=a2[:], in0=a1[:][:, :, 0:3], in1=a1[:][:, :, 3:6], op=op)
                            pl = small.tile([128, P], BF, name=f"pl_{m2}_{r}", bufs=2)
                            plv = pl[:].rearrange("p (i k) -> p i k", k=1)
                            nc.vector.tensor_tensor(
                                out=plv, in0=a2[:][:, :, 0:1], in1=a2[:][:, :, 1:2], op=op)
                            nc.vector.tensor_tensor(
                                out=plv, in0=plv, in1=a2[:][:, :, 2:3], op=op)
                            pooled[r + m2] = pl

                    # output: relu(pooled @ Wp + bp), count-0 rows -> relu(bp)
                    ps_o = psO.tile([P, D], FP, tag="pso", name="pso")
                    for kc in range(4):
                        nc.tensor.matmul(
                            ps_o[:], pooled[kc][:], wp_bf[:, kc * D : (kc + 1) * D],
                            start=(kc == 0), stop=False)
                    nc.tensor.matmul(ps_o[:], ones_bf, bp_bf, start=False, stop=True)
                    out_sb = outsp.tile([P, D], FP, name="outsb")
                    nc.scalar.activation(out=out_sb[:], in_=ps_o[:], func=ACTF.Relu, scale=st["has"][:])
                    nc.vector.scalar_tensor_tensor(
                        out=out2all[:, s * D : (s + 1) * D], in0=rbp_sb,
                        scalar=st["hinv"][:], in1=out_sb[:],
                        op0=ALU.mult, op1=ALU.add)

                # skewed emission: selection (front_a) runs TWO scenes ahead
                # of the MLP (front_b) so the index-roundtrip DMA latency of
                # scene s hides under scene s+1's geometry/selection; scene
                # s-1's tail is emitted AFTER front_b(s) so is_equal(s) and
                # statq never queue behind pooling; statq(s+1) is prefetched
                # mid-B(s)
                stas = [front_a(0), front_a(1), front_a(2)] if S > 2 else \
                    [front_a(s) for s in range(S)]
                if rep == 0:
                    # bulk weight loads on the idle Pool SWDGE queue, after
                    # the pj broadcasts so scene-0 geometry isn't delayed
                    for sx in sexts:
                        nc.gpsimd.dma_start(
                            out=sx[P : 2 * P, :],
                            in_=bass.AP(tensor=wpack.tensor,
                                        offset=wpack.offset + W_EYE,
                                        ap=[[WCOLS, P], [1, PK]]))
                    nc.gpsimd.dma_start(out=wsb[:, W_WM2:W_EYE],
                                        in_=wpack[:, W_WM2:W_EYE])
                statq_box = [statq_mm(0)]
                for s in range(S):
                    nxt = [s + 1] if s + 1 < S else None
                    statq_cur = statq_box.pop()
                    tails.append(front_b(stas[s], statq_cur, nxt))
                    if nxt is not None:
                        statq_box.append(nxt.pop())
                    if s >= 1:
                        back(tails[s - 1])
                    if s + 3 < S:
                        stas.append(front_a(s + 3))
                    if s == S - 1:
                        # scenes 0..S-2 ship mid-stream; only the last
                        # scene's slice remains in the drain tail
                        nc.sync.dma_start(
                            out=bass.AP(tensor=outp.tensor, offset=outp.offset,
                                        ap=[[D, P], [P * D, S - 1], [1, D]]),
                            in_=out2all[:, 0 : (S - 1) * D].rearrange(
                                "p (s d) -> p s d", d=D))
                back(tails[S - 1])
                nc.sync.dma_start(
                    out=bass.AP(tensor=outp.tensor,
                                offset=outp.offset + (S - 1) * P * D,
                                ap=[[D, P], [1, D]]),
                    in_=out2all[:, (S - 1) * D : S * D])

    nc.finalize()
    return nc


def _host_prep(h_states, seq_start_end, end_pos, end_velocity, before_end_pos,
               W_s, b_s, W_v, b_v, Wm1, bm1, Wm2, bm2, Wp, bp):
    """Fold weights (f64) and pack per-core input maps."""
    f64 = np.float64
    bf16 = ml_dtypes.bfloat16
    A = np.concatenate(
        [W_s.astype(f64) @ Wm1[:E].astype(f64),
         W_v.astype(f64) @ Wm1[E : 2 * E].astype(f64)], axis=0
    ).astype(np.float32)                                      # (4, 512)
    beff = (bm1.astype(f64) + b_s.astype(f64) @ Wm1[:E].astype(f64)
            + b_v.astype(f64) @ Wm1[E : 2 * E].astype(f64)).astype(np.float32)
    Wm1h = np.ascontiguousarray(Wm1[2 * E :])                 # (128, 512)

    wm2p = np.ascontiguousarray(
        Wm2.reshape(4, 128, D).transpose(1, 0, 2).reshape(128, 4 * D))
    wpp = np.ascontiguousarray(
        Wp.reshape(4, 128, D).transpose(1, 0, 2).reshape(128, 4 * D))
    beff_pack = np.ascontiguousarray(beff.reshape(4, 128).T)  # (128, 4)
    bm2_pack = np.ascontiguousarray(bm2.reshape(2, 128).T)    # (128, 2)

    iota = (64.0 - np.arange(P, dtype=np.float32))[None, :] * (
        1.0 - np.eye(P, dtype=np.float32))
    iotj = np.arange(P, dtype=np.float32).reshape(P, 1)

    pos = end_pos.reshape(B_SEQ, P, 2)
    vel = end_velocity.reshape(B_SEQ, P, 2)
    bef = before_end_pos.reshape(B_SEQ, P, 2)
    hid = h_states.reshape(B_SEQ, P, H)

    in_maps = []
    for c in range(NCORES):
        sl = slice(c * S, (c + 1) * S)
        p_, v_, b_ = pos[sl], vel[sl], bef[sl]          # (S, P, 2)
        geo = np.zeros((8, NP_CORE), np.float32)
        geo[0] = p_[..., 0].reshape(-1)
        geo[1] = p_[..., 1].reshape(-1)
        geo[2] = v_[..., 0].reshape(-1)
        geo[3] = v_[..., 1].reshape(-1)
        geo[4] = b_[..., 0].reshape(-1)
        geo[5] = b_[..., 1].reshape(-1)
        geoT = np.ascontiguousarray(
            geo.reshape(8, S, P).transpose(2, 1, 0).reshape(P, S * 8))
        hidT = np.ascontiguousarray(hid[sl].reshape(NP_CORE, H).T)  # (128, 256)

        geopack = np.zeros((128, GCOLS), np.float32)
        geopack[0:P, G_GEOT : G_GEOT + 8 * S] = geoT
        geopack[0:P, G_IOTA : G_IOTA + P] = iota
        geopack[0:P, G_IOTJ : G_IOTJ + 1] = iotj
        geopack[0:P, G_RBP : G_RBP + D] = np.maximum(bp, 0.0)[None, :]
        geopack[:, G_BM2 : G_BM2 + 2] = bm2_pack
        geopack[0, G_PXY : G_PXY + 2 * NP_CORE] = (
            geo[0:2].reshape(2, S, P).transpose(1, 0, 2).reshape(-1))

        wpk = np.zeros((128, WCOLS), np.float32)
        wpk[0:H, W_HIDT : W_HIDT + NP_CORE] = hidT
        wpk[0:4, W_A4 : W_A4 + MLP] = A
        wpk[4, W_A4 : W_A4 + MLP] = beff
        wpk[0:H, W_WM1H : W_WM1H + MLP] = Wm1h
        wpk[:, W_WM2 : W_WM2 + 4 * D] = wm2p
        wpk[:, W_WP : W_WP + 4 * D] = wpp
        wpk[0:1, W_BP : W_BP + D] = bp.reshape(1, D)
        wpk[0:1, W_ONES : W_ONES + P] = 1.0
        wpk[0:P, W_EYE : W_EYE + PK] = np.repeat(
            -np.eye(P, dtype=np.float32), K, axis=1)
        wpk[0:4, W_G4 : W_G4 + NP_CORE] = geo[0:4]
        wpk[4, W_G4 : W_G4 + NP_CORE] = 1.0
        in_maps.append({"geopack": geopack, "wpack": wpk.astype(bf16)})
    return in_maps


_CACHED_NC = None


def kernel(**inputs):
    global _CACHED_NC
    inputs = {k: np.asarray(v) for k, v in inputs.items()}
    in_maps = _host_prep(**inputs)
    if _CACHED_NC is None:
        _CACHED_NC = build_program()
    res = run_bass_kernel_spmd(_CACHED_NC, in_maps, core_ids=list(range(NCORES)))
    out = np.concatenate([r["outp"] for r in res.results], axis=0)
    return out.astype(np.float32)


if __name__ == "__main__":
    np.random.seed(0)
    fake = {
        "h_states": np.random.randn(1, N, H).astype(np.float32),
        "seq_start_end": np.stack(
            [np.arange(B_SEQ, dtype=np.int32) * P,
             (np.arange(B_SEQ, dtype=np.int32) + 1) * P], axis=1),
        "end_pos": (np.random.rand(N, 2) * 8).astype(np.float32),
        "end_velocity": (0.5 * np.random.randn(N, 2)).astype(np.float32),
        "before_end_pos": np.random.randn(N, 2).astype(np.float32),
        "W_s": np.random.randn(2, E).astype(np.float32) * 0.5,
        "b_s": np.random.randn(E).astype(np.float32) * 0.5,
        "W_v": np.random.randn(2, E).astype(np.float32) * 0.5,
        "b_v": np.random.randn(E).astype(np.float32) * 0.5,
        "Wm1": (np.random.randn(2 * E + H, MLP) / 16).astype(np.float32),
        "bm1": (np.random.randn(MLP) / 16).astype(np.float32),
        "Wm2": (np.random.randn(MLP, D) / 22).astype(np.float32),
        "bm2": (np.random.randn(D) / 22).astype(np.float32),
        "Wp": (np.random.randn(2 * D, D) / 22).astype(np.float32),
        "bp": (np.random.randn(D) / 22).astype(np.float32),
    }
    out = kernel(**fake)
    print("kernel ran, out", out.shape, out.dtype, float(np.abs(out).max()))
